# revision 24
# baseline (speedup 1.0000x reference)
"""Trainium2 Bass kernel for nn_CaptioningRNN (attention-LSTM over T=128 steps).

Sharding: tensor-parallel over the 4H gate dimension across 8 NeuronCores.
Core j owns H-slice j (128 h-rows) of each of the 4 gates, so the per-step
LSTM cell state (c, h) for that slice lives entirely on core j.

Attention restructure: attn_t @ Wattn = sum_l w_t[n,l] * (A[:,:,l] @ Wattn)[n,:],
so P[n, l, :] = A[n, :, l] @ Wattn_slice + b is precomputed once (constant;
bias folded in since softmax weights sum to 1), and per step the rank-16
attention contribution is added into the same PSUM accumulation as the
h-ktiles via 8 PE matmuls whose K=128 stationary packs diag(w_l) for two
l values (l and l+8) stacked on the partition axis. The softmax runs on
128 partitions with the score row duplicated onto both halves (same
per-lane cost as 64) so both diagonal blocks can be built lane-locally.
One collective per step (hT slices + score partials); the next step's
x @ Wx k-tiles are issued last on the PE so they execute inside that
collective's window. Score partials use a DVE multiply + add-tree with
unit-stride inner dims. All activations go through Tanh (sigmoid =
(1+tanh(x/2))/2, softmax exp via the tanh identity) so ScalarE keeps one
activation table loaded.
"""
import numpy as np
import ml_dtypes

import bass_rust
import concourse.bass as bass
import concourse.mybir as mybir
from concourse import tile
from concourse.alu_op_type import AluOpType
from concourse.bass_utils import run_bass_kernel_spmd

BF16 = ml_dtypes.bfloat16
F32 = mybir.dt.float32
BF = mybir.dt.bfloat16
AF = mybir.ActivationFunctionType
AX = mybir.AxisListType

N, T, D, H, L, R = 64, 128, 512, 1024, 16, 8
HS, GS = H // R, 4 * H // R  # 128, 512
L2 = L // 2
SCALE = 1.0 / np.sqrt(H)


def _split_waits(nc, cap=1):
    """Walrus here rejects >cap sync waits per instruction; hoist extras
    onto preceding same-engine NOPs."""
    ctr = 0
    for fn in nc.m.functions:
        for bb in fn.blocks:
            out, changed = [], False
            for ins in bb.instructions:
                si = ins.sync_info
                if si is not None and si.on_wait and len(si.on_wait) > cap:
                    waits = list(si.on_wait)
                    extra, keep = waits[:-cap], waits[-cap:]
                    for i in range(0, len(extra), cap):
                        out.append(bass_rust.InstNoOp(
                            name=f"zz_waitsplit_{ctr}", engine=ins.engine,
                            sync_info=bass_rust.SyncInfo(
                                on_wait=extra[i:i + cap], on_update=[])))
                        ctr += 1
                    ins.sync_info = bass_rust.SyncInfo(
                        on_wait=keep, on_update=list(si.on_update or []))
                    changed = True
                out.append(ins)
            if changed:
                bb.instructions = out
    return ctr


def _prep_inputs(x, A, Wx, Wh, Wattn, b):
    x = np.asarray(x, np.float32)
    A_flat = np.asarray(A, np.float32).reshape(N, H, L)
    Wx = np.asarray(Wx, np.float32)
    Wh = np.asarray(Wh, np.float32)
    Wattn = np.asarray(Wattn, np.float32)
    b = np.asarray(b, np.float32)

    W_comb = np.concatenate([Wh, Wattn], axis=0)
    h0 = A_flat.mean(axis=2).astype(np.float32)
    # UNscaled initial scores; SCALE is folded into the softmax activation.
    scores0 = np.einsum('nh,nhl->nl', h0, A_flat).astype(np.float32)
    xT = np.ascontiguousarray(
        x.transpose(1, 2, 0).reshape(T, 4, 128, N)).astype(BF16)
    h0T = np.ascontiguousarray(
        h0.T.reshape(8, 128, N).transpose(1, 0, 2)).astype(BF16)
    # asTf[p, c, l, n] = A_flat[n, c*128+p, l]  (full-H transposed A)
    asTf = np.ascontiguousarray(
        A_flat.transpose(1, 2, 0).reshape(8, 128, L, N).transpose(1, 0, 2, 3)
    ).astype(BF16)
    dmask2 = np.concatenate([np.eye(N, dtype=np.float32),
                             np.eye(N, dtype=np.float32)], axis=0)
    # bottom softmax half sees l rotated by 8 so w_bf[64+j, k] = w[j, k+8]
    s02 = np.concatenate([scores0, np.roll(scores0, -8, axis=1)],
                         axis=0).astype(np.float32)

    in_maps = []
    for j in range(R):
        cols = np.array([g * H + j * HS + i for g in range(4) for i in range(HS)])
        hsl = slice(j * HS, (j + 1) * HS)
        in_maps.append({
            "xT": xT,
            "whaj": np.ascontiguousarray(
                W_comb[:, cols].reshape(16, 128, GS)).astype(BF16),
            "wxj": np.ascontiguousarray(
                Wx[:, cols].reshape(4, 128, GS)).astype(BF16),
            "brep": np.tile(b[cols], (128, 1)).astype(np.float32),
            "asTf": asTf,
            "anml": np.ascontiguousarray(
                A_flat[:, hsl, :].transpose(0, 2, 1)).astype(BF16),
            "dmask2": dmask2,
            "iden": np.eye(128, dtype=np.float32),
            "h0T": h0T,
            "c0": np.ascontiguousarray(h0[:, hsl]),
            "s02": s02,
        })
    return in_maps


def _build():
    nc = bass.Bass("TRN2", target_bir_lowering=False, debug=False, num_devices=R)
    rg = [list(range(R))]

    xT_d = nc.dram_tensor("xT", [T, 4, 128, N], BF, kind="ExternalInput")
    whaj_d = nc.dram_tensor("whaj", [16, 128, GS], BF, kind="ExternalInput")
    wxj_d = nc.dram_tensor("wxj", [4, 128, GS], BF, kind="ExternalInput")
    brep_d = nc.dram_tensor("brep", [128, GS], F32, kind="ExternalInput")
    asTf_d = nc.dram_tensor("asTf", [128, 8, L, N], BF, kind="ExternalInput")
    anml_d = nc.dram_tensor("anml", [N, L, HS], BF, kind="ExternalInput")
    dmask2_d = nc.dram_tensor("dmask2", [128, N], F32, kind="ExternalInput")
    iden_d = nc.dram_tensor("iden", [128, 128], F32, kind="ExternalInput")
    h0T_d = nc.dram_tensor("h0T", [128, 8, N], BF, kind="ExternalInput")
    c0_d = nc.dram_tensor("c0", [N, HS], F32, kind="ExternalInput")
    s02_d = nc.dram_tensor("s02", [128, L], F32, kind="ExternalInput")
    out_d = nc.dram_tensor("out", [N, T, HS], F32, kind="ExternalOutput")

    with tile.TileContext(nc) as tc:
        with tc.tile_pool(name="const", bufs=1) as cp, \
             tc.tile_pool(name="state", bufs=1) as st, \
             tc.tile_pool(name="dram", bufs=2, space="DRAM") as dp:

            whaj = cp.tile([128, 16, GS], BF, name="whaj")
            wxj = cp.tile([128, 4, GS], BF, name="wxj")
            brep = cp.tile([128, GS], F32, name="brep")
            nc.sync.dma_start(out=wxj[:, :, :], in_=wxj_d.rearrange("k p g -> p k g"))
            nc.sync.dma_start(out=brep[:, :], in_=brep_d[:, :])
            asTf = cp.tile([128, 8, L, N], BF, name="asTf")
            anml = cp.tile([N, L, HS], BF, name="anml")
            dmask2 = cp.tile([128, N], F32, name="dmask2")
            iden = cp.tile([128, 128], F32, name="iden")
            nc.sync.dma_start(out=whaj[:, :, :], in_=whaj_d.rearrange("k p g -> p k g"))
            nc.sync.dma_start(out=asTf[:, :, :, :], in_=asTf_d[:, :, :, :])
            nc.sync.dma_start(out=anml[:, :, :], in_=anml_d[:, :, :])
            nc.sync.dma_start(out=dmask2[:, :], in_=dmask2_d[:, :])
            nc.sync.dma_start(out=iden[:, :], in_=iden_d[:, :])

            c = st.tile([N, HS], F32, name="c")
            nc.sync.dma_start(out=c[:, :], in_=c0_d[:, :])
            # P2[p, k, :]: rows 0:64 hold P_l (l=k) + b, rows 64:128 hold
            # P_l (l=k+8) + b
            P2 = st.tile([128, L2, GS], BF, name="P2")

            # ---- one-time: P[n, l, :] = A[n, :, l] @ Wattn_slice + b ----
            with tc.tile_pool(name="ppre", bufs=2, space="PSUM") as pq:
                for k in range(L2):
                    pp = pq.tile([128, GS], F32, name="pp", tag="pp")
                    for cch in range(8):
                        nc.tensor.matmul(pp[0:64, :], asTf[:, cch, k, :],
                                         whaj[:, 8 + cch, :],
                                         start=(cch == 0), stop=(cch == 7))
                    for cch in range(8):
                        nc.tensor.matmul(pp[64:128, :], asTf[:, cch, k + L2, :],
                                         whaj[:, 8 + cch, :],
                                         start=(cch == 0), stop=(cch == 7))
                    with nc.allow_low_precision(reason="P2 stored bf16"):
                        nc.vector.tensor_add(out=P2[:, k, :], in0=pp[:, :],
                                             in1=brep[:, :])

            with tc.tile_pool(name="wk", bufs=2) as wk, \
                 tc.tile_pool(name="ps_a", bufs=2, space="PSUM") as ps_a, \
                 tc.tile_pool(name="ps_t", bufs=2, space="PSUM") as ps_t:

                hT_full = wk.tile([128, 8, N], BF, name="hT0", tag="hT_full")
                nc.sync.dma_start(out=hT_full[:, :, :], in_=h0T_d[:, :, :])
                scores2 = wk.tile([128, L], F32, name="scores0", tag="scores2")
                nc.sync.dma_start(out=scores2[:, :], in_=s02_d[:, :])

                # prologue: x k-tiles for t=0 seed the t=0 PSUM accumulation
                xtile = wk.tile([128, 4, N], BF, name="xtile", tag="xtile")
                nc.sync.dma_start(out=xtile[:, :, :],
                                  in_=xT_d[0].rearrange("k p n -> p k n"))
                pa = ps_a.tile([N, GS], F32, name="pa", tag="pa")
                for kt in range(4):
                    nc.tensor.matmul(pa[:, :], xtile[:, kt, :], wxj[:, kt, :],
                                     start=(kt == 0), stop=False)

                sparts2 = None
                for t in range(T):
                    if t > 0:
                        # scores_t = sum_r spart_r, duplicated on both halves
                        scores2 = wk.tile([128, L], F32, name="scores",
                                          tag="scores2")
                        nc.vector.reduce_sum(
                            out=scores2[:, :],
                            in_=sparts2.rearrange("p r l -> p l r"), axis=AX.X)
                    # prefetch next x tile early (consumed at end of step)
                    if t < T - 1:
                        xtile_n = wk.tile([128, 4, N], BF, name="xtile",
                                          tag="xtile")
                        nc.sync.dma_start(
                            out=xtile_n[:, :, :],
                            in_=xT_d[t + 1].rearrange("k p n -> p k n"))
                    # softmax over 16 on 128 partitions (scores bounded; SCALE
                    # folded into tanh scale): exp(x) = (1+tanh(x/2))/(1-tanh(x/2))
                    th = wk.tile([128, L], F32, name="th", tag="th")
                    nc.scalar.activation(th[:, :], scores2[:, :], AF.Tanh,
                                         scale=0.5 * SCALE)
                    den = wk.tile([128, L], F32, name="den", tag="den")
                    nc.vector.tensor_scalar(out=den[:, :], in0=th[:, :],
                                            scalar1=-1.0, scalar2=1.0,
                                            op0=AluOpType.mult, op1=AluOpType.add)
                    rden = wk.tile([128, L], F32, name="rden", tag="rden")
                    nc.vector.reciprocal(out=rden[:, :], in_=den[:, :])
                    e = wk.tile([128, L], F32, name="e", tag="e")
                    se = wk.tile([128, 1], F32, name="se", tag="se")
                    nc.vector.scalar_tensor_tensor(
                        out=e[:, :], in0=th[:, :], scalar=1.0, in1=rden[:, :],
                        op0=AluOpType.add, op1=AluOpType.mult,
                        accum_out=se[:, :])
                    rse = wk.tile([128, 1], F32, name="rse", tag="rse")
                    nc.vector.reciprocal(out=rse[:, :], in_=se[:, :])
                    # packed diag stationaries: rows 0:64 diag(w_l), rows
                    # 64:128 diag(w_{l+8}) — bottom softmax half is l-rotated
                    # by 8. Softmax normalization (rse) fused into the build;
                    # two halves so the first matmuls start sooner.
                    wd2 = wk.tile([128, L2, N], BF, name="wd2", tag="wd2")
                    nc.vector.scalar_tensor_tensor(
                        out=wd2[:, 0:4, :],
                        in0=e[:, 0:4, None].broadcast_to((128, 4, N)),
                        scalar=rse[:, 0:1],
                        in1=dmask2[:, None, :].broadcast_to((128, 4, N)),
                        op0=AluOpType.mult, op1=AluOpType.mult)
                    nc.vector.scalar_tensor_tensor(
                        out=wd2[:, 4:L2, :],
                        in0=e[:, 4:L2, None].broadcast_to((128, 4, N)),
                        scalar=rse[:, 0:1],
                        in1=dmask2[:, None, :].broadcast_to((128, 4, N)),
                        op0=AluOpType.mult, op1=AluOpType.mult)

                    # gate pre-activations: 8 h-ktiles + 8 packed diag(w)@P2
                    # k-tiles accumulate onto the x@Wx seed already in PSUM
                    for kt in range(8):
                        nc.tensor.matmul(pa[:, :], hT_full[:, kt, :],
                                         whaj[:, kt, :],
                                         start=False, stop=False)
                    for k in range(L2):
                        nc.tensor.matmul(pa[:, :], wd2[:, k, :], P2[:, k, :],
                                         start=False, stop=(k == L2 - 1))

                    sg3 = wk.tile([N, 3 * HS], F32, name="sg3", tag="sg3")
                    nc.scalar.activation(sg3[:, :], pa[:, 0:3 * HS], AF.Tanh,
                                         scale=0.5)
                    sig = wk.tile([N, 3 * HS], F32, name="sig", tag="sig")
                    nc.vector.tensor_scalar(out=sig[:, :], in0=sg3[:, :],
                                            scalar1=1.0, scalar2=0.5,
                                            op0=AluOpType.add, op1=AluOpType.mult)
                    gt = wk.tile([N, HS], F32, name="gt", tag="gt")
                    nc.scalar.activation(gt[:, :], pa[:, 3 * HS:4 * HS], AF.Tanh)
                    t1 = wk.tile([N, HS], F32, name="t1", tag="t1")
                    nc.vector.tensor_mul(out=t1[:, :], in0=sig[:, 0:HS],
                                         in1=gt[:, :])
                    nc.vector.tensor_mul(out=c[:, :], in0=sig[:, HS:2 * HS],
                                         in1=c[:, :])
                    nc.vector.tensor_add(out=c[:, :], in0=c[:, :], in1=t1[:, :])
                    tanc = wk.tile([N, HS], F32, name="tanc", tag="tanc")
                    nc.scalar.activation(tanc[:, :], c[:, :], AF.Tanh)
                    h_j = wk.tile([N, HS], F32, name="h_j", tag="h_j")
                    nc.vector.tensor_mul(out=h_j[:, :], in0=sig[:, 2 * HS:3 * HS],
                                         in1=tanc[:, :])
                    nc.sync.dma_start(out=out_d[:, t, :], in_=h_j[:, :])
                    if t == T - 1:
                        break

                    # hT for the gather (PE transpose) + local score partials,
                    # packed into one staging tile so a single DMA sends both
                    pt2 = ps_t.tile([128, N], F32, name="pt2", tag="pt2")
                    nc.tensor.transpose(pt2[:, :], h_j[:, :], iden[0:N, 0:N])
                    stg = wk.tile([128, N + L], BF, name="stg", tag="stg")
                    nc.vector.tensor_copy(out=stg[:, 0:N], in_=pt2[:, :])
                    nc.vector.memset(stg[64:128, N:N + L], 0)

                    # spart[n, l] = sum_h h_j[n, h] * anml[n, l, h] via DVE
                    # multiply + add-tree over the unit-stride inner h axis
                    h_bf = wk.tile([N, HS], BF, name="h_bf", tag="h_bf")
                    nc.vector.tensor_copy(out=h_bf[:, :], in_=h_j[:, :])
                    sp0 = wk.tile([N, L, HS], BF, name="sp0", tag="sp0")
                    nc.vector.tensor_tensor(
                        out=sp0[:, :, :], in0=anml[:, :, :],
                        in1=h_bf[:, None, :].broadcast_to((N, L, HS)),
                        op=AluOpType.mult)
                    v1 = wk.tile([N, L, 64], BF, name="v1", tag="v1")
                    nc.vector.tensor_add(out=v1[:, :, :], in0=sp0[:, :, 0:64],
                                         in1=sp0[:, :, 64:128])
                    v2 = wk.tile([N, L, 32], BF, name="v2", tag="v2")
                    nc.vector.tensor_add(out=v2[:, :, :], in0=v1[:, :, 0:32],
                                         in1=v1[:, :, 32:64])
                    v3 = wk.tile([N, L, 16], BF, name="v3", tag="v3")
                    nc.vector.tensor_add(out=v3[:, :, :], in0=v2[:, :, 0:16],
                                         in1=v2[:, :, 16:32])
                    v4 = wk.tile([N, L, 8], BF, name="v4", tag="v4")
                    nc.vector.tensor_add(out=v4[:, :, :], in0=v3[:, :, 0:8],
                                         in1=v3[:, :, 8:16])
                    with nc.allow_low_precision(reason="score partials in bf16"):
                        nc.vector.reduce_sum(out=stg[0:N, N:N + L],
                                             in_=v4[:, :, :], axis=AX.X)

                    sendA = dp.tile([128 * (N + L)], BF, name="sendA",
                                    tag="sendA")
                    recvA = dp.tile([R, 128 * (N + L)], BF, name="recvA",
                                    tag="recvA", addr_space="Shared")
                    nc.sync.dma_start(
                        out=sendA[:].rearrange("(p q) -> p q", p=128),
                        in_=stg[:, :])
                    nc.gpsimd.collective_compute(
                        "AllGather", AluOpType.bypass, replica_groups=rg,
                        ins=[sendA[:].opt()], outs=[recvA[:, :].opt()])

                    # next step's x k-tiles execute inside the gather window,
                    # seeding the next PSUM accumulation
                    pa = ps_a.tile([N, GS], F32, name="pa", tag="pa")
                    for kt in range(4):
                        nc.tensor.matmul(pa[:, :], xtile_n[:, kt, :],
                                         wxj[:, kt, :],
                                         start=(kt == 0), stop=False)
                    xtile = xtile_n

                    # sparts first: they head the next step's critical chain
                    sparts2 = wk.tile([128, 8, L], BF, name="sparts2",
                                      tag="sparts2")
                    rview = recvA[:, :].rearrange("r (p q) -> p r q", p=128)
                    nc.sync.dma_start(
                        out=sparts2[0:64, :, :], in_=rview[0:64, :, N:N + L])
                    # bottom half with l rotated by 8 (two halves swapped)
                    nc.sync.dma_start(
                        out=sparts2[64:128, :, 0:L2],
                        in_=rview[0:64, :, N + L2:N + L])
                    nc.sync.dma_start(
                        out=sparts2[64:128, :, L2:L],
                        in_=rview[0:64, :, N:N + L2])
                    hT_full = wk.tile([128, 8, N], BF, name="hT_full",
                                      tag="hT_full")
                    nc.sync.dma_start(
                        out=hT_full[:, 0:4, :],
                        in_=recvA[0:4, :].rearrange("r (p q) -> p r q",
                                                    p=128)[:, :, 0:N])
                    nc.sync.dma_start(
                        out=hT_full[:, 4:8, :],
                        in_=recvA[4:8, :].rearrange("r (p q) -> p r q",
                                                    p=128)[:, :, 0:N])

    _split_waits(nc, cap=1)
    return nc


_NC_CACHE = None


def kernel(**inputs) -> np.ndarray:
    global _NC_CACHE
    in_maps = _prep_inputs(**inputs)
    if _NC_CACHE is None:
        _NC_CACHE = _build()
    res = run_bass_kernel_spmd(_NC_CACHE, in_maps, core_ids=list(range(R)))
    out = np.zeros((N, T, H), dtype=np.float32)
    for j, r in enumerate(res.results):
        out[:, :, j * HS:(j + 1) * HS] = np.asarray(r["out"]).reshape(N, T, HS)
    return out
